# revision 1
# baseline (speedup 1.0000x reference)
"""AEC-OLS multi-frame filter kernel for 8 NeuronCores.

Data-parallel over batch B=128 -> 16 independent AEC instances per core.
The antialias op (ifft -> keep last `hop` time samples -> fft) is a linear
circulant operator A on the 4096 frequency bins; it is precomputed on host
and applied on-device as real matmuls (TensorEngine), so no device FFT is
needed. All device math is real f32 (re/im planes); complex64 outputs are
assembled on host.
"""

import numpy as np

B, F, W, C = 128, 16, 4096, 1
HOP = 2048
N_CORES = 8
B_LOC = B // N_CORES  # 16


def _antialias_matrix():
    # antialias(x) = FFT(mask * IFFT(x)); mask keeps last HOP time samples.
    # Time-domain masking == circular convolution in frequency domain:
    #   A[k, l] = c[(k - l) mod W],  c = FFT(mask) / W.
    mask = (np.arange(W) >= (W - HOP)).astype(np.float64)
    c = np.fft.fft(mask) / W
    idx = (np.arange(W)[:, None] - np.arange(W)[None, :]) % W
    A = c[idx]  # (W, W) complex, A @ x == antialias(x)
    AT = A.T.copy()
    return AT.real.astype(np.float32), AT.imag.astype(np.float32)


_AT_RE, _AT_IM = None, None


def _get_AT():
    global _AT_RE, _AT_IM
    if _AT_RE is None:
        _AT_RE, _AT_IM = _antialias_matrix()
    return _AT_RE, _AT_IM


def _compute_core_np(ur, ui, dr, di, wr, wi, ar, ai):
    # ur/ui/wr/wi: (b, F, W)   dr/di: (b, W)   ar/ai: (W, W) == A.T parts
    y_re = np.einsum("bfw,bfw->bw", wr, ur) - np.einsum("bfw,bfw->bw", wi, ui)
    y_im = np.einsum("bfw,bfw->bw", wr, ui) + np.einsum("bfw,bfw->bw", wi, ur)
    out_re = dr - y_re
    out_im = di - y_im
    yaa_re = y_re @ ar - y_im @ ai
    yaa_im = y_re @ ai + y_im @ ar
    e_re = out_re @ ar - out_im @ ai
    e_im = out_re @ ai + out_im @ ar
    g_re = ur * e_re[:, None, :] + ui * e_im[:, None, :]
    g_im = ur * e_im[:, None, :] - ui * e_re[:, None, :]
    loss = 0.5 * np.sum(e_re * e_re + e_im * e_im, axis=1)
    return out_re, out_im, e_re, e_im, yaa_re, yaa_im, g_re, g_im, loss


def _run_numpy(ur, ui, dr, di, wr, wi):
    ar, ai = _get_AT()
    return _compute_core_np(ur, ui, dr, di, wr, wi, ar, ai)


_PMAP_FN = None


def _get_pmap():
    global _PMAP_FN
    if _PMAP_FN is not None:
        return _PMAP_FN
    import jax
    import jax.numpy as jnp

    def f(ur, ui, dr, di, wr, wi, ar, ai):
        y_re = jnp.sum(wr * ur - wi * ui, axis=1)
        y_im = jnp.sum(wr * ui + wi * ur, axis=1)
        out_re = dr - y_re
        out_im = di - y_im
        yaa_re = y_re @ ar - y_im @ ai
        yaa_im = y_re @ ai + y_im @ ar
        e_re = out_re @ ar - out_im @ ai
        e_im = out_re @ ai + out_im @ ar
        g_re = ur * e_re[:, None, :] + ui * e_im[:, None, :]
        g_im = ur * e_im[:, None, :] - ui * e_re[:, None, :]
        loss = 0.5 * jnp.sum(e_re * e_re + e_im * e_im, axis=1)
        return out_re, out_im, e_re, e_im, yaa_re, yaa_im, g_re, g_im, loss

    _PMAP_FN = jax.pmap(f, in_axes=(0, 0, 0, 0, 0, 0, None, None))
    return _PMAP_FN


def kernel(u_re, u_im, d_re, d_im, w_re, w_im, hop_size):
    u_re = np.asarray(u_re, np.float32).reshape(B, F, W)
    u_im = np.asarray(u_im, np.float32).reshape(B, F, W)
    w_re = np.asarray(w_re, np.float32).reshape(B, F, W)
    w_im = np.asarray(w_im, np.float32).reshape(B, F, W)
    d_re = np.asarray(d_re, np.float32).reshape(B, W)
    d_im = np.asarray(d_im, np.float32).reshape(B, W)

    # shard batch across the 8 cores
    sh = lambda x: x.reshape((N_CORES, B_LOC) + x.shape[1:])
    ar, ai = _get_AT()

    try:
        pf = _get_pmap()
        res = pf(sh(u_re), sh(u_im), sh(d_re), sh(d_im), sh(w_re), sh(w_im), ar, ai)
        res = [np.asarray(r) for r in res]
        out_re, out_im, e_re, e_im, yaa_re, yaa_im, g_re, g_im, loss = [
            r.reshape((-1,) + r.shape[2:]) for r in res
        ]
    except Exception:
        out_re, out_im, e_re, e_im, yaa_re, yaa_im, g_re, g_im, loss = _run_numpy(
            u_re, u_im, d_re, d_im, w_re, w_im
        )

    out = (out_re + 1j * out_im).astype(np.complex64).reshape(B, W, C)
    e = (e_re + 1j * e_im).astype(np.complex64).reshape(B, W, C)
    y_aa = (yaa_re + 1j * yaa_im).astype(np.complex64).reshape(B, W, C)
    grad = (g_re + 1j * g_im).astype(np.complex64).reshape(B, F, W, C)
    loss = loss.astype(np.float32).reshape(B)
    return out, e, y_aa, grad, loss
